# revision 20
# baseline (speedup 1.0000x reference)
"""CrossNetMix (DCN-V2 mixture-of-low-rank-experts) Trainium2 kernel.

Data-parallel over batch across 8 cores (2048 rows each); feature-major
([d, b]) on chip so every matmul contraction lands on SBUF partitions.

Matmul stages (gating, V, C, U) run in fp8-e4m3 DoubleRow mode. The 4
batch chunks (512 cols each) are processed in interleaved pairs at the
PASS level: each weight tile is loaded once and streamed over both
chunks of the pair (the second matmul sets ldweights=False so hardware
skips the redundant weight reload). PSUM: 2 manually-sliced misc banks
host gating/softmax outputs for both halves; 4 banks rotate V/C/wbp
outputs; 2 banks rotate U outputs.

Residual reformulation: T_i = sum_{j<=i} uv_j + 1 + B_i (B = bias
cumsum), so xi_i = T_i (.) x0 -- no identity matmuls. GPSIMD/Pool
cannot touch PSUM on TRN2, so each U-stage PSUM tile is first spilled
to SBUF bf16 with a single copy (ACT for half A, DVE for half B); the
Pool engine then accumulates T in place (scalar_tensor_tensor,
SBUF-only) and the xi = T (.) x0 multiply is split Pool/DVE (fp8 for
the next layer's matmuls, bf16 on the last layer for the output DMA),
overlapping the U-stage tail and the next layer's gating/V matmuls.
The softmax gate broadcast stays a bf16 matmul; ys = tanh(C h1) (.) w
runs on DVE (reads PSUM). Scalar engine: exp, tanh, half the spills.
"""

import os
import sys

import ml_dtypes
import numpy as np

if "/opt/trn_rl_repo" not in sys.path:
    sys.path.insert(0, "/opt/trn_rl_repo")

import concourse.bass as bass
import concourse.bacc as bacc
import concourse.mybir as mybir
from concourse.tile import TileContext
from concourse.bass_utils import run_bass_kernel_spmd

AF = mybir.ActivationFunctionType
OP = mybir.AluOpType
DR = mybir.MatmulPerfMode.DoubleRow
F32 = mybir.dt.float32
WDT = mybir.dt.bfloat16
F8 = mybir.dt.float8e4
BF16 = ml_dtypes.bfloat16
NPF8 = ml_dtypes.float8_e4m3

N_CROSS = 3
E = 8            # experts
D = 1024         # feature dim
R = 64           # low rank
B = 16384        # full batch
NCORES = 8
BC = B // NCORES  # rows per core
CHUNK = 512       # batch tile (matmul free dim)
NCHUNK = BC // CHUNK
P = 128
KC = D // P       # d-chunks
ER = E * R        # 512
MC = ER // P      # (e,r)-chunks

LDW_SKIP = False  # second matmul of a same-weights pair skips LdWeights
DEBUG = False     # dump per-stage intermediates for pair 0


def _build():
    nc = bacc.Bacc(None)
    dbg = {}
    if DEBUG:
        dbg["gA"] = nc.declare_dram_parameter("dbg_gA", [2 * E, CHUNK], F32,
                                              isOutput=True)
        dbg["eA"] = nc.declare_dram_parameter("dbg_eA", [E, CHUNK], WDT,
                                              isOutput=True)
        dbg["wA"] = nc.declare_dram_parameter("dbg_wA", [E, CHUNK], WDT,
                                              isOutput=True)
        dbg["yA"] = nc.declare_dram_parameter("dbg_yA", [P, MC, CHUNK], F8,
                                              isOutput=True)
        dbg["t0A"] = nc.declare_dram_parameter("dbg_t0A", [P, KC, CHUNK], WDT,
                                               isOutput=True)
        dbg["t0B"] = nc.declare_dram_parameter("dbg_t0B", [P, KC, CHUNK], WDT,
                                               isOutput=True)
        dbg["x1A"] = nc.declare_dram_parameter("dbg_x1A", [P, KC, CHUNK], F8,
                                               isOutput=True)
        dbg["t1A"] = nc.declare_dram_parameter("dbg_t1A", [P, KC, CHUNK], WDT,
                                               isOutput=True)
        dbg["t1B"] = nc.declare_dram_parameter("dbg_t1B", [P, KC, CHUNK], WDT,
                                               isOutput=True)
    xT = nc.declare_dram_parameter("xT", [D, BC], WDT, isOutput=False)
    xT8 = nc.declare_dram_parameter("xT8", [D, BC], F8, isOutput=False)
    Vl = nc.declare_dram_parameter("Vl", [N_CROSS, D, ER], F8, isOutput=False)
    Cb = nc.declare_dram_parameter("Cb", [N_CROSS, MC, 2, P, P], F8, isOutput=False)
    Ul = nc.declare_dram_parameter("Ul", [N_CROSS, ER, D], F8, isOutput=False)
    # gating weights padded to 16 cols: DoubleRow lhsT outer step must be
    # a multiple of 16 (s3_lw dual-fp8 restriction)
    WgT = nc.declare_dram_parameter("WgT", [D, 2 * E], F8, isOutput=False)
    # bTd[i, p, kc]: layer-i per-partition combine scalar:
    #   i == 0: 1 + b[0];  i > 0: b[i]
    bTd = nc.declare_dram_parameter("bTd", [N_CROSS, P, KC], F32, isOutput=False)
    sel = nc.declare_dram_parameter("sel", [E, MC + 1, P], WDT, isOutput=False)
    outT = nc.declare_dram_parameter("outT", [D, BC], WDT, isOutput=True)

    def mm_pair(ps_a, ps_b, w, rhs_a, rhs_b, start, stop, perf_mode=None):
        nc.tensor.matmul(ps_a, w, rhs_a, start=start, stop=stop,
                         perf_mode=perf_mode)
        i2 = nc.tensor.matmul(ps_b, w, rhs_b, start=start, stop=stop,
                              perf_mode=perf_mode)
        if LDW_SKIP:
            i2.ins.ldweights = False

    with TileContext(nc) as tc:
        with (
            tc.sbuf_pool(name="wpool", bufs=1) as wpool,
            tc.sbuf_pool(name="xpool", bufs=NCHUNK) as xpool,
            tc.sbuf_pool(name="x8pool", bufs=NCHUNK) as x8pool,
            tc.sbuf_pool(name="tpool", bufs=NCHUNK) as tpool,
            tc.sbuf_pool(name="xopool", bufs=2) as xopool,
            tc.sbuf_pool(name="h1pool", bufs=2) as h1pool,
            tc.sbuf_pool(name="h2pool", bufs=4) as h2pool,
            tc.sbuf_pool(name="ypool", bufs=2) as ypool,
            tc.sbuf_pool(name="uspool", bufs=4) as uspool,
            tc.sbuf_pool(name="spool", bufs=2) as spool,
            tc.psum_pool(name="psmisc", bufs=1) as psmisc,
            tc.psum_pool(name="psmm", bufs=4) as psmm,
            tc.psum_pool(name="psu", bufs=2) as psu,
        ):
            xTr = xT.rearrange("(kc p) b -> p kc b", p=P)
            xT8r = xT8.rearrange("(kc p) b -> p kc b", p=P)
            outr = outT.rearrange("(kc p) b -> p kc b", p=P)
            Vlr = Vl.rearrange("i (kc p) m -> p i kc m", p=P)
            Ulr = Ul.rearrange("i (mc p) d -> p i mc d", p=P)
            Cbr = Cb.rearrange("i m j p s -> p i m j s")

            def alloc_x(c):
                x0 = xpool.tile([P, KC, CHUNK], WDT, tag="x0", name=f"x0_{c}")
                s8 = x8pool.tile([P, KC, CHUNK], F8, tag="s8", name=f"s8_{c}")
                T = tpool.tile([P, KC, CHUNK], WDT, tag="T", name=f"T_{c}")
                return x0, s8, T

            def load_s8(tt, c):
                cbs = slice(c * CHUNK, (c + 1) * CHUNK)
                nc.sync.dma_start(tt[1], xT8r[:, :, cbs])

            def load_x0(tt, c):
                cbs = slice(c * CHUNK, (c + 1) * CHUNK)
                nc.sync.dma_start(tt[0], xTr[:, :, cbs])

            wg_sb = wpool.tile([P, KC, 2 * E], F8)
            nc.scalar.dma_start(wg_sb, WgT.rearrange("(kc p) e -> p kc e", p=P))

            v_sb = wpool.tile([P, N_CROSS, KC, ER], F8)
            u_sb = wpool.tile([P, N_CROSS, MC, D], F8)
            c_sb = wpool.tile([P, N_CROSS, MC, 2, P], F8)
            b_sb = wpool.tile([P, N_CROSS, KC], F32)

            nc.scalar.dma_start(v_sb[:, 0, 0:KC // 2], Vlr[:, 0, 0:KC // 2])
            nc.scalar.dma_start(v_sb[:, 0, KC // 2:], Vlr[:, 0, KC // 2:])
            nc.scalar.dma_start(c_sb, Cbr)

            # two misc PSUM banks host the small per-half matmul outputs.
            # HW requires matmul outputs at partition base 0, so gps -> sums
            # -> wps reuse the same partitions sequentially (each stage only
            # starts after the previous one's consumer read it, which the
            # data flow already forces).
            misc1 = psmisc.tile([P, CHUNK], F32, name="misc1")
            misc2 = psmisc.tile([P, CHUNK], F32, name="misc2")

            # q1 in need-order: s8 c0/c1 (gating+V), U0, x0 c0/c1 (combine),
            # U1, U2; prefetch of chunks 2/3 is emitted inside the loop.
            tiles = {0: alloc_x(0), 1: alloc_x(1)}
            load_s8(tiles[0], 0)
            load_s8(tiles[1], 1)
            nc.sync.dma_start(u_sb[:, 0], Ulr[:, 0])
            load_x0(tiles[0], 0)
            sel_sb = wpool.tile([E, MC + 1, P], WDT)
            nc.scalar.dma_start(sel_sb, sel[:])
            nc.scalar.dma_start(v_sb[:, 1], Vlr[:, 1])
            load_x0(tiles[1], 1)
            nc.sync.dma_start(u_sb[:, 1], Ulr[:, 1])
            nc.scalar.dma_start(b_sb, bTd.rearrange("i p kc -> p i kc"))
            nc.scalar.dma_start(v_sb[:, 2], Vlr[:, 2])
            nc.sync.dma_start(u_sb[:, 2], Ulr[:, 2])

            ones_col = sel_sb[:, MC, 0:1]     # [E, 1] ones (sums lhsT)
            ones_row = sel_sb[0:1, MC, 0:E]   # [1, E] ones (wps lhsT)

            for pr in range(NCHUNK // 2):
                ca, cb = 2 * pr, 2 * pr + 1
                for i in range(N_CROSS):
                    if i == 1 and cb + 2 < NCHUNK:
                        for c in (ca + 2, cb + 2):
                            tiles[c] = alloc_x(c)
                            load_s8(tiles[c], c)
                            load_x0(tiles[c], c)
                    x0A, s8A, TA = tiles[ca]
                    x0B, s8B, TB = tiles[cb]
                    last = i == N_CROSS - 1
                    if last:
                        xoA = xopool.tile([P, KC, CHUNK], WDT, tag="xo",
                                          name=f"xoA_{pr}")
                        xoB = xopool.tile([P, KC, CHUNK], WDT, tag="xo",
                                          name=f"xoB_{pr}")
                    # ---- gating (fp8 DoubleRow, 16-col padded) ----
                    gpsA = misc1[0:2 * E, :]
                    gpsB = misc2[0:2 * E, :]
                    for q in range(KC // 2):
                        mm_pair(gpsA, gpsB,
                                wg_sb[:, 2 * q: 2 * q + 2, :],
                                s8A[:, 2 * q: 2 * q + 2, :],
                                s8B[:, 2 * q: 2 * q + 2, :],
                                start=(q == 0), stop=(q == KC // 2 - 1),
                                perf_mode=DR)
                    expA = spool.tile([E, CHUNK], WDT, tag="expA")
                    expB = spool.tile([E, CHUNK], WDT, tag="expB")
                    nc.scalar.activation(expA, gpsA[0:E, :], AF.Exp)
                    nc.scalar.activation(expB, gpsB[0:E, :], AF.Exp)
                    # ---- V stage (fp8 DoubleRow), softmax tail woven in ----
                    h1A = h1pool.tile([P, MC, CHUNK], F8, tag="h1A")
                    h1B = h1pool.tile([P, MC, CHUNK], F8, tag="h1B")
                    for mc in range(MC):
                        vpsA = psmm.tile([P, CHUNK], F32, tag="mm")
                        vpsB = psmm.tile([P, CHUNK], F32, tag="mm")
                        for q in range(KC // 2):
                            mm_pair(vpsA, vpsB,
                                    v_sb[:, i, 2 * q: 2 * q + 2,
                                         mc * P: (mc + 1) * P],
                                    s8A[:, 2 * q: 2 * q + 2, :],
                                    s8B[:, 2 * q: 2 * q + 2, :],
                                    start=(q == 0), stop=(q == KC // 2 - 1),
                                    perf_mode=DR)
                        nc.scalar.activation(h1A[:, mc, :], vpsA, AF.Tanh)
                        nc.scalar.activation(h1B[:, mc, :], vpsB, AF.Tanh)
                        if mc == 0:
                            # softmax denominators (PE waits on exp only)
                            sumsA = misc1[0:1, :]
                            sumsB = misc2[0:1, :]
                            mm_pair(sumsA, sumsB, ones_col, expA, expB,
                                    start=True, stop=True)
                            rfA = spool.tile([1, CHUNK], F32, tag="rfA")
                            rfB = spool.tile([1, CHUNK], F32, tag="rfB")
                            nc.vector.reciprocal_approx_fast(rfA, sumsA)
                            nc.vector.reciprocal_approx_fast(rfB, sumsB)
                            rrA = spool.tile([1, CHUNK], WDT, tag="rrA")
                            rrB = spool.tile([1, CHUNK], WDT, tag="rrB")
                            nc.vector.tensor_copy(rrA, rfA)
                            nc.vector.tensor_copy(rrB, rfB)
                        if mc == 2:
                            # broadcast 1/sum to E partitions (rrow ready now)
                            wpsA = misc1[0:E, :]
                            wpsB = misc2[0:E, :]
                            mm_pair(wpsA, wpsB, ones_row, rrA, rrB,
                                    start=True, stop=True)
                            wsbA = spool.tile([E, CHUNK], WDT, tag="wsbA")
                            wsbB = spool.tile([E, CHUNK], WDT, tag="wsbB")
                            nc.vector.tensor_tensor(wsbA, expA, wpsA, OP.mult)
                            nc.vector.tensor_tensor(wsbB, expB, wpsB, OP.mult)
                    if DEBUG and pr == 0 and i == 0:
                        gcp = spool.tile([2 * E, CHUNK], F32, tag="gcp")
                        nc.scalar.activation(gcp, gpsA, AF.Copy)
                        nc.sync.dma_start(dbg["gA"][:], gcp)
                        nc.sync.dma_start(dbg["eA"][:], expA)
                        nc.sync.dma_start(dbg["wA"][:], wsbA)
                    # ---- C stage + gate broadcast + ys (Pool) ----
                    ysA = ypool.tile([P, MC, CHUNK], F8, tag="ysA")
                    ysB = ypool.tile([P, MC, CHUNK], F8, tag="ysB")
                    for mc in range(MC):
                        qb = (mc // 2) * 2
                        cpsA = psmm.tile([P, CHUNK], F32, tag="mm")
                        cpsB = psmm.tile([P, CHUNK], F32, tag="mm")
                        mm_pair(cpsA, cpsB, c_sb[:, i, mc, :, :],
                                h1A[:, qb: qb + 2, :], h1B[:, qb: qb + 2, :],
                                start=True, stop=True, perf_mode=DR)
                        wbpA = psmm.tile([P, CHUNK], F32, tag="mm")
                        wbpB = psmm.tile([P, CHUNK], F32, tag="mm")
                        mm_pair(wbpA, wbpB, sel_sb[:, mc, :], wsbA, wsbB,
                                start=True, stop=True)
                        h2A = h2pool.tile([P, CHUNK], F32, tag="h2")
                        h2B = h2pool.tile([P, CHUNK], F32, tag="h2")
                        nc.scalar.activation(h2A, cpsA, AF.Tanh)
                        nc.scalar.activation(h2B, cpsB, AF.Tanh)
                        nc.vector.tensor_tensor(ysA[:, mc, :], h2A, wbpA,
                                                OP.mult)
                        nc.vector.tensor_tensor(ysB[:, mc, :], h2B, wbpB,
                                                OP.mult)
                    if DEBUG and pr == 0 and i == 0:
                        nc.sync.dma_start(dbg["yA"][:], ysA)
                    # ---- U stage + spill + T accumulate (Pool) + xi ----
                    for dc in range(KC):
                        upsA = psu.tile([P, CHUNK], F32, tag="u")
                        upsB = psu.tile([P, CHUNK], F32, tag="u")
                        for q in range(MC // 2):
                            mm_pair(upsA, upsB,
                                    u_sb[:, i, 2 * q: 2 * q + 2,
                                         dc * P: (dc + 1) * P],
                                    ysA[:, 2 * q: 2 * q + 2, :],
                                    ysB[:, 2 * q: 2 * q + 2, :],
                                    start=(q == 0), stop=(q == MC // 2 - 1),
                                    perf_mode=DR)
                        dlt = b_sb[:, i, dc: dc + 1]
                        if i == 0:
                            # spill straight into T with the delta folded in
                            nc.scalar.activation(TA[:, dc, :], upsA,
                                                 AF.Identity, bias=dlt)
                            nc.vector.tensor_scalar(TB[:, dc, :], upsB, dlt,
                                                    None, OP.add)
                        else:
                            usA = uspool.tile([P, CHUNK], WDT, tag="us")
                            usB = uspool.tile([P, CHUNK], WDT, tag="us")
                            nc.scalar.activation(usA, upsA, AF.Identity,
                                                 bias=dlt)
                            nc.vector.tensor_scalar(usB, upsB, dlt,
                                                    None, OP.add)
                            nc.gpsimd.tensor_tensor(
                                TA[:, dc, :], usA, TA[:, dc, :], OP.add)
                            nc.gpsimd.tensor_tensor(
                                TB[:, dc, :], usB, TB[:, dc, :], OP.add)
                        outA = (xoA if last else s8A)[:, dc, :]
                        outB = (xoB if last else s8B)[:, dc, :]
                        nc.vector.tensor_tensor(
                            outA, TA[:, dc, :], x0A[:, dc, :], OP.mult)
                        nc.vector.tensor_tensor(
                            outB, TB[:, dc, :], x0B[:, dc, :], OP.mult)
                        if last:
                            bsA = slice(ca * CHUNK, (ca + 1) * CHUNK)
                            bsB = slice(cb * CHUNK, (cb + 1) * CHUNK)
                            eng = nc.sync if dc % 2 == 0 else nc.scalar
                            eng.dma_start(outr[:, dc, bsA], xoA[:, dc, :])
                            eng2 = nc.scalar if dc % 2 == 0 else nc.sync
                            eng2.dma_start(outr[:, dc, bsB], xoB[:, dc, :])
                    if DEBUG and pr == 0 and i == 0:
                        nc.sync.dma_start(dbg["t0A"][:], TA)
                        nc.sync.dma_start(dbg["t0B"][:], TB)
                        nc.sync.dma_start(dbg["x1A"][:], s8A)
                    if DEBUG and pr == 0 and i == 1:
                        nc.sync.dma_start(dbg["t1A"][:], TA)
                        nc.sync.dma_start(dbg["t1B"][:], TB)
    nc.compile()
    return nc


_CTX = {}


def _get_nc():
    if "nc" not in _CTX:
        _CTX["nc"] = _build()
    return _CTX["nc"]


def _prep_weights(U, V, C, Wg, b):
    f = np.float32
    U = np.asarray(U, dtype=f)
    V = np.asarray(V, dtype=f)
    C = np.asarray(C, dtype=f)
    Wg = np.asarray(Wg, dtype=f)
    b = np.asarray(b, dtype=f)
    # Vl[i, d, e*R+r] = V[i, e, d, r]
    Vl = np.ascontiguousarray(V.transpose(0, 2, 1, 3).reshape(N_CROSS, D, ER))
    # Ul[i, e*R+r, d] = U[i, e, d, r]
    Ul = np.ascontiguousarray(U.transpose(0, 1, 3, 2).reshape(N_CROSS, ER, D))
    # DoubleRow C: out-block mc pairs rhs h1 blocks (qb, qb+1); the plane
    # matching block mc carries the block-diag expert pair, the other is 0.
    Cb2 = np.zeros((N_CROSS, MC, 2, P, P), dtype=f)
    for i in range(N_CROSS):
        for m in range(MC):
            blk = np.zeros((P, P), dtype=f)
            blk[:R, :R] = C[i, 2 * m]
            blk[R:, R:] = C[i, 2 * m + 1]
            Cb2[i, m, m % 2] = blk
    WgT = np.zeros((D, 2 * E), dtype=f)
    WgT[:, :E] = Wg.T
    # bTd[i, p, kc]: layer-0 carries 1 + b[0]; later layers just b[i]
    bd = b.copy()
    bd[0] = 1.0 + b[0]
    bTd = np.ascontiguousarray(bd.reshape(N_CROSS, KC, P).transpose(0, 2, 1))
    sel = np.zeros((E, MC + 1, P), dtype=f)
    for m in range(MC):
        for j in range(P):
            sel[2 * m + j // R, m, j] = 1.0
    sel[:, MC, :] = 1.0
    return dict(
        Vl=Vl.astype(NPF8),
        Ul=Ul.astype(NPF8),
        Cb=Cb2.astype(NPF8),
        WgT=WgT.astype(NPF8),
        bTd=bTd,
        sel=sel.astype(BF16),
    )


def kernel(x, U, V, C, Wg, b, _trace=False):
    nc = _get_nc()
    w = _prep_weights(U, V, C, Wg, b)
    xs = np.asarray(x, dtype=np.float32).reshape(NCORES, BC, D)
    in_maps = []
    for ci in range(NCORES):
        xt = np.ascontiguousarray(xs[ci].T)
        m = {"xT": xt.astype(BF16), "xT8": xt.astype(NPF8)}
        m.update(w)
        in_maps.append(m)
    res = run_bass_kernel_spmd(nc, in_maps, list(range(NCORES)), trace=_trace)
    kernel.last_result = res
    out = np.concatenate(
        [np.asarray(res.results[ci]["outT"]).astype(np.float32).T
         for ci in range(NCORES)],
        axis=0,
    )
    return np.ascontiguousarray(out, dtype=np.float32)


# revision 21
# speedup vs baseline: 1.0914x; 1.0914x over previous
"""CrossNetMix (DCN-V2 mixture-of-low-rank-experts) Trainium2 kernel.

Data-parallel over batch across 8 cores (2048 rows each); feature-major
([d, b]) on chip so every matmul contraction lands on SBUF partitions.

Matmul stages (gating, V, C, U) run in fp8-e4m3 DoubleRow mode. The 4
batch chunks (512 cols each) are processed in interleaved pairs at the
PASS level: each weight tile is loaded once and streamed over both
chunks of the pair (the second matmul sets ldweights=False so hardware
skips the redundant weight reload). PSUM: 2 misc banks host gating /
softmax-sum / gate-broadcast outputs (all at partition base 0, reused
sequentially -- the data flow already serializes them); 4 banks rotate
V/C/wbp outputs; 2 banks rotate U outputs.

Residual reformulation: with S_i = sum_{j<i} (uv_j + b_j),
  xi_i = x0 (.) (S_i + 1).
Each layer's U matmuls accumulate uv into PSUM; for layers > 0 an
identity matmul adds the previous S (bf16, SBUF) into the same
accumulation (the PE absorbs the add -- the elementwise engines have no
spare capacity at this pace). Then one DVE scalar_tensor_tensor per
d-chunk emits xi = (S + (1 + B_i)) (.) x0 in fp8 for the next layer's
matmuls (bf16 on the last layer for the output DMA), and an ACT copy
spills S back to SBUF for the next layer.
"""

import os
import sys

import ml_dtypes
import numpy as np

if "/opt/trn_rl_repo" not in sys.path:
    sys.path.insert(0, "/opt/trn_rl_repo")

import concourse.bass as bass
import concourse.bacc as bacc
import concourse.mybir as mybir
from concourse.tile import TileContext
from concourse.bass_utils import run_bass_kernel_spmd

AF = mybir.ActivationFunctionType
OP = mybir.AluOpType
DR = mybir.MatmulPerfMode.DoubleRow
F32 = mybir.dt.float32
WDT = mybir.dt.bfloat16
F8 = mybir.dt.float8e4
BF16 = ml_dtypes.bfloat16
NPF8 = ml_dtypes.float8_e4m3

N_CROSS = 3
E = 8            # experts
D = 1024         # feature dim
R = 64           # low rank
B = 16384        # full batch
NCORES = 8
BC = B // NCORES  # rows per core
CHUNK = 512       # batch tile (matmul free dim)
NCHUNK = BC // CHUNK
P = 128
KC = D // P       # d-chunks
ER = E * R        # 512
MC = ER // P      # (e,r)-chunks

LDW_SKIP = True   # second matmul of a same-weights pair skips LdWeights


def _build():
    nc = bacc.Bacc(None)
    xT = nc.declare_dram_parameter("xT", [D, BC], WDT, isOutput=False)
    xT8 = nc.declare_dram_parameter("xT8", [D, BC], F8, isOutput=False)
    Vl = nc.declare_dram_parameter("Vl", [N_CROSS, D, ER], F8, isOutput=False)
    Cb = nc.declare_dram_parameter("Cb", [N_CROSS, MC, 2, P, P], F8, isOutput=False)
    Ul = nc.declare_dram_parameter("Ul", [N_CROSS, ER, D], F8, isOutput=False)
    # gating weights padded to 16 cols: DoubleRow lhsT outer step must be
    # a multiple of 16 (s3_lw dual-fp8 restriction)
    WgT = nc.declare_dram_parameter("WgT", [D, 2 * E], F8, isOutput=False)
    # bTc[i, p, kc] = 1 + sum_{j<=i} b[j, kc*P+p]  (per-partition stt scalar)
    bTc = nc.declare_dram_parameter("bTc", [N_CROSS, P, KC], F32, isOutput=False)
    sel = nc.declare_dram_parameter("sel", [E, MC + 1, P], WDT, isOutput=False)
    id128 = nc.declare_dram_parameter("id128", [P, P], WDT, isOutput=False)
    outT = nc.declare_dram_parameter("outT", [D, BC], WDT, isOutput=True)

    def mm_pair(ps_a, ps_b, w, rhs_a, rhs_b, start, stop, perf_mode=None,
                skip_b_ldw=True):
        nc.tensor.matmul(ps_a, w, rhs_a, start=start, stop=stop,
                         perf_mode=perf_mode)
        i2 = nc.tensor.matmul(ps_b, w, rhs_b, start=start, stop=stop,
                              perf_mode=perf_mode)
        if LDW_SKIP and skip_b_ldw:
            i2.ins.ldweights = False

    with TileContext(nc) as tc:
        with (
            tc.sbuf_pool(name="wpool", bufs=1) as wpool,
            tc.sbuf_pool(name="xpool", bufs=NCHUNK) as xpool,
            tc.sbuf_pool(name="x8pool", bufs=NCHUNK) as x8pool,
            tc.sbuf_pool(name="spool2", bufs=NCHUNK) as spool2,
            tc.sbuf_pool(name="xopool", bufs=2) as xopool,
            tc.sbuf_pool(name="h1pool", bufs=2) as h1pool,
            tc.sbuf_pool(name="h2pool", bufs=4) as h2pool,
            tc.sbuf_pool(name="ypool", bufs=2) as ypool,
            tc.sbuf_pool(name="spool", bufs=2) as spool,
            tc.psum_pool(name="psmisc", bufs=1) as psmisc,
            tc.psum_pool(name="psmm", bufs=4) as psmm,
            tc.psum_pool(name="psu", bufs=2) as psu,
        ):
            xTr = xT.rearrange("(kc p) b -> p kc b", p=P)
            xT8r = xT8.rearrange("(kc p) b -> p kc b", p=P)
            outr = outT.rearrange("(kc p) b -> p kc b", p=P)
            Vlr = Vl.rearrange("i (kc p) m -> p i kc m", p=P)
            Ulr = Ul.rearrange("i (mc p) d -> p i mc d", p=P)
            Cbr = Cb.rearrange("i m j p s -> p i m j s")

            def alloc_x(c):
                x0 = xpool.tile([P, KC, CHUNK], WDT, tag="x0", name=f"x0_{c}")
                s8 = x8pool.tile([P, KC, CHUNK], F8, tag="s8", name=f"s8_{c}")
                S = spool2.tile([P, KC, CHUNK], WDT, tag="S", name=f"S_{c}")
                return x0, s8, S

            def load_s8(tt, c):
                cbs = slice(c * CHUNK, (c + 1) * CHUNK)
                nc.sync.dma_start(tt[1], xT8r[:, :, cbs])

            def load_x0(tt, c):
                cbs = slice(c * CHUNK, (c + 1) * CHUNK)
                nc.sync.dma_start(tt[0], xTr[:, :, cbs])

            wg_sb = wpool.tile([P, KC, 2 * E], F8)
            nc.scalar.dma_start(wg_sb, WgT.rearrange("(kc p) e -> p kc e", p=P))

            v_sb = wpool.tile([P, N_CROSS, KC, ER], F8)
            u_sb = wpool.tile([P, N_CROSS, MC, D], F8)
            c_sb = wpool.tile([P, N_CROSS, MC, 2, P], F8)
            b_sb = wpool.tile([P, N_CROSS, KC], F32)
            id_sb = wpool.tile([P, P], WDT)

            nc.scalar.dma_start(v_sb[:, 0, 0:KC // 2], Vlr[:, 0, 0:KC // 2])
            nc.scalar.dma_start(v_sb[:, 0, KC // 2:], Vlr[:, 0, KC // 2:])
            nc.scalar.dma_start(c_sb, Cbr)

            # two misc PSUM banks host the small per-half matmul outputs.
            # HW requires matmul outputs at partition base 0, so gps -> sums
            # -> wps reuse the same partitions sequentially (the data flow
            # already forces that order).
            misc1 = psmisc.tile([P, CHUNK], F32, name="misc1")
            misc2 = psmisc.tile([P, CHUNK], F32, name="misc2")

            # q1 in need-order: s8 c0/c1 (gating+V), U0, id, x0 c0/c1
            # (combine), U1, U2; chunk 2/3 prefetch is emitted in the loop.
            tiles = {0: alloc_x(0), 1: alloc_x(1)}
            load_s8(tiles[0], 0)
            load_s8(tiles[1], 1)
            nc.sync.dma_start(u_sb[:, 0], Ulr[:, 0])
            nc.sync.dma_start(id_sb, id128[:])
            load_x0(tiles[0], 0)
            sel_sb = wpool.tile([E, MC + 1, P], WDT)
            nc.scalar.dma_start(sel_sb, sel[:])
            nc.scalar.dma_start(v_sb[:, 1], Vlr[:, 1])
            load_x0(tiles[1], 1)
            nc.sync.dma_start(u_sb[:, 1], Ulr[:, 1])
            nc.scalar.dma_start(b_sb, bTc.rearrange("i p kc -> p i kc"))
            nc.scalar.dma_start(v_sb[:, 2], Vlr[:, 2])
            nc.sync.dma_start(u_sb[:, 2], Ulr[:, 2])

            ones_col = sel_sb[:, MC, 0:1]     # [E, 1] ones (sums lhsT)
            ones_row = sel_sb[0:1, MC, 0:E]   # [1, E] ones (wps lhsT)

            for pr in range(NCHUNK // 2):
                ca, cb = 2 * pr, 2 * pr + 1
                for i in range(N_CROSS):
                    if i == 1 and cb + 2 < NCHUNK:
                        for c in (ca + 2, cb + 2):
                            tiles[c] = alloc_x(c)
                            load_s8(tiles[c], c)
                            load_x0(tiles[c], c)
                    x0A, s8A, SA = tiles[ca]
                    x0B, s8B, SB = tiles[cb]
                    last = i == N_CROSS - 1
                    if last:
                        xoA = xopool.tile([P, KC, CHUNK], WDT, tag="xo",
                                          name=f"xoA_{pr}")
                        xoB = xopool.tile([P, KC, CHUNK], WDT, tag="xo",
                                          name=f"xoB_{pr}")
                    # ---- gating (fp8 DoubleRow, 16-col padded) ----
                    gpsA = misc1[0:2 * E, :]
                    gpsB = misc2[0:2 * E, :]
                    for q in range(KC // 2):
                        mm_pair(gpsA, gpsB,
                                wg_sb[:, 2 * q: 2 * q + 2, :],
                                s8A[:, 2 * q: 2 * q + 2, :],
                                s8B[:, 2 * q: 2 * q + 2, :],
                                start=(q == 0), stop=(q == KC // 2 - 1),
                                perf_mode=DR)
                    expA = spool.tile([E, CHUNK], WDT, tag="expA")
                    expB = spool.tile([E, CHUNK], WDT, tag="expB")
                    nc.scalar.activation(expA, gpsA[0:E, :], AF.Exp)
                    nc.scalar.activation(expB, gpsB[0:E, :], AF.Exp)
                    # ---- V stage (fp8 DoubleRow), softmax tail woven in ----
                    h1A = h1pool.tile([P, MC, CHUNK], F8, tag="h1A")
                    h1B = h1pool.tile([P, MC, CHUNK], F8, tag="h1B")
                    for mc in range(MC):
                        vpsA = psmm.tile([P, CHUNK], F32, tag="mm")
                        vpsB = psmm.tile([P, CHUNK], F32, tag="mm")
                        for q in range(KC // 2):
                            mm_pair(vpsA, vpsB,
                                    v_sb[:, i, 2 * q: 2 * q + 2,
                                         mc * P: (mc + 1) * P],
                                    s8A[:, 2 * q: 2 * q + 2, :],
                                    s8B[:, 2 * q: 2 * q + 2, :],
                                    start=(q == 0), stop=(q == KC // 2 - 1),
                                    perf_mode=DR)
                        nc.scalar.activation(h1A[:, mc, :], vpsA, AF.Tanh)
                        nc.scalar.activation(h1B[:, mc, :], vpsB, AF.Tanh)
                        if mc == 0:
                            # softmax denominators (PE waits on exp only)
                            sumsA = misc1[0:1, :]
                            sumsB = misc2[0:1, :]
                            mm_pair(sumsA, sumsB, ones_col, expA, expB,
                                    start=True, stop=True)
                            rfA = spool.tile([1, CHUNK], F32, tag="rfA")
                            rfB = spool.tile([1, CHUNK], F32, tag="rfB")
                            nc.vector.reciprocal_approx_fast(rfA, sumsA)
                            nc.vector.reciprocal_approx_fast(rfB, sumsB)
                            rrA = spool.tile([1, CHUNK], WDT, tag="rrA")
                            rrB = spool.tile([1, CHUNK], WDT, tag="rrB")
                            nc.gpsimd.tensor_copy(rrA, rfA)
                            nc.gpsimd.tensor_copy(rrB, rfB)
                        if mc == 2:
                            # broadcast 1/sum to E partitions (rrow ready now)
                            wpsA = misc1[0:E, :]
                            wpsB = misc2[0:E, :]
                            mm_pair(wpsA, wpsB, ones_row, rrA, rrB,
                                    start=True, stop=True)
                            wsbA = spool.tile([E, CHUNK], WDT, tag="wsbA")
                            wsbB = spool.tile([E, CHUNK], WDT, tag="wsbB")
                            nc.vector.tensor_tensor(wsbA, expA, wpsA, OP.mult)
                            nc.vector.tensor_tensor(wsbB, expB, wpsB, OP.mult)
                    # ---- C stage + gate broadcast + ys ----
                    ysA = ypool.tile([P, MC, CHUNK], F8, tag="ysA")
                    ysB = ypool.tile([P, MC, CHUNK], F8, tag="ysB")
                    for mc in range(MC):
                        qb = (mc // 2) * 2
                        cpsA = psmm.tile([P, CHUNK], F32, tag="mm")
                        cpsB = psmm.tile([P, CHUNK], F32, tag="mm")
                        mm_pair(cpsA, cpsB, c_sb[:, i, mc, :, :],
                                h1A[:, qb: qb + 2, :], h1B[:, qb: qb + 2, :],
                                start=True, stop=True, perf_mode=DR)
                        wbpA = psmm.tile([P, CHUNK], F32, tag="mm")
                        wbpB = psmm.tile([P, CHUNK], F32, tag="mm")
                        mm_pair(wbpA, wbpB, sel_sb[:, mc, :], wsbA, wsbB,
                                start=True, stop=True)
                        h2A = h2pool.tile([P, CHUNK], F32, tag="h2")
                        h2B = h2pool.tile([P, CHUNK], F32, tag="h2")
                        nc.scalar.activation(h2A, cpsA, AF.Tanh)
                        nc.scalar.activation(h2B, cpsB, AF.Tanh)
                        nc.vector.tensor_tensor(ysA[:, mc, :], h2A, wbpA,
                                                OP.mult)
                        nc.vector.tensor_tensor(ysB[:, mc, :], h2B, wbpB,
                                                OP.mult)
                    # ---- U stage + S accumulate (PE) + combine ----
                    for dc in range(KC):
                        upsA = psu.tile([P, CHUNK], F32, tag="u")
                        upsB = psu.tile([P, CHUNK], F32, tag="u")
                        for q in range(MC // 2):
                            mm_pair(upsA, upsB,
                                    u_sb[:, i, 2 * q: 2 * q + 2,
                                         dc * P: (dc + 1) * P],
                                    ysA[:, 2 * q: 2 * q + 2, :],
                                    ysB[:, 2 * q: 2 * q + 2, :],
                                    start=(q == 0),
                                    stop=(i == 0 and q == MC // 2 - 1),
                                    perf_mode=DR)
                        if i > 0:
                            mm_pair(upsA, upsB, id_sb,
                                    SA[:, dc, :], SB[:, dc, :],
                                    start=False, stop=True)
                        # xi = (S + (1 + B_i)) * x0
                        dlt = b_sb[:, i, dc: dc + 1]
                        outA = (xoA if last else s8A)[:, dc, :]
                        outB = (xoB if last else s8B)[:, dc, :]
                        nc.vector.scalar_tensor_tensor(
                            outA, upsA, dlt, x0A[:, dc, :], OP.add, OP.mult)
                        nc.vector.scalar_tensor_tensor(
                            outB, upsB, dlt, x0B[:, dc, :], OP.add, OP.mult)
                        if not last:
                            nc.scalar.activation(SA[:, dc, :], upsA, AF.Copy)
                            nc.scalar.activation(SB[:, dc, :], upsB, AF.Copy)
                        else:
                            bsA = slice(ca * CHUNK, (ca + 1) * CHUNK)
                            bsB = slice(cb * CHUNK, (cb + 1) * CHUNK)
                            eng = nc.sync if dc % 2 == 0 else nc.scalar
                            eng.dma_start(outr[:, dc, bsA], xoA[:, dc, :])
                            eng2 = nc.scalar if dc % 2 == 0 else nc.sync
                            eng2.dma_start(outr[:, dc, bsB], xoB[:, dc, :])
    nc.compile()
    return nc


_CTX = {}


def _get_nc():
    if "nc" not in _CTX:
        _CTX["nc"] = _build()
    return _CTX["nc"]


def _prep_weights(U, V, C, Wg, b):
    f = np.float32
    U = np.asarray(U, dtype=f)
    V = np.asarray(V, dtype=f)
    C = np.asarray(C, dtype=f)
    Wg = np.asarray(Wg, dtype=f)
    b = np.asarray(b, dtype=f)
    # Vl[i, d, e*R+r] = V[i, e, d, r]
    Vl = np.ascontiguousarray(V.transpose(0, 2, 1, 3).reshape(N_CROSS, D, ER))
    # Ul[i, e*R+r, d] = U[i, e, d, r]
    Ul = np.ascontiguousarray(U.transpose(0, 1, 3, 2).reshape(N_CROSS, ER, D))
    # DoubleRow C: out-block mc pairs rhs h1 blocks (qb, qb+1); the plane
    # matching block mc carries the block-diag expert pair, the other is 0.
    Cb2 = np.zeros((N_CROSS, MC, 2, P, P), dtype=f)
    for i in range(N_CROSS):
        for m in range(MC):
            blk = np.zeros((P, P), dtype=f)
            blk[:R, :R] = C[i, 2 * m]
            blk[R:, R:] = C[i, 2 * m + 1]
            Cb2[i, m, m % 2] = blk
    WgT = np.zeros((D, 2 * E), dtype=f)
    WgT[:, :E] = Wg.T
    # bTc[i, p, kc] = 1 + cumsum_i b  (stt per-partition scalar)
    bc = 1.0 + np.cumsum(b, axis=0)
    bTc = np.ascontiguousarray(bc.reshape(N_CROSS, KC, P).transpose(0, 2, 1))
    sel = np.zeros((E, MC + 1, P), dtype=f)
    for m in range(MC):
        for j in range(P):
            sel[2 * m + j // R, m, j] = 1.0
    sel[:, MC, :] = 1.0
    return dict(
        Vl=Vl.astype(NPF8),
        Ul=Ul.astype(NPF8),
        Cb=Cb2.astype(NPF8),
        WgT=WgT.astype(NPF8),
        bTc=bTc,
        sel=sel.astype(BF16),
        id128=np.eye(P, dtype=f).astype(BF16),
    )


def kernel(x, U, V, C, Wg, b, _trace=False):
    nc = _get_nc()
    w = _prep_weights(U, V, C, Wg, b)
    xs = np.asarray(x, dtype=np.float32).reshape(NCORES, BC, D)
    in_maps = []
    for ci in range(NCORES):
        xt = np.ascontiguousarray(xs[ci].T)
        m = {"xT": xt.astype(BF16), "xT8": xt.astype(NPF8)}
        m.update(w)
        in_maps.append(m)
    res = run_bass_kernel_spmd(nc, in_maps, list(range(NCORES)), trace=_trace)
    kernel.last_result = res
    out = np.concatenate(
        [np.asarray(res.results[ci]["outT"]).astype(np.float32).T
         for ci in range(NCORES)],
        axis=0,
    )
    return np.ascontiguousarray(out, dtype=np.float32)


# revision 28
# speedup vs baseline: 1.2198x; 1.1176x over previous
"""CrossNetMix (DCN-V2 mixture-of-low-rank-experts) Trainium2 kernel.

Data-parallel over batch across 8 cores (2048 rows each); feature-major
([d, b]) on chip so every matmul contraction lands on SBUF partitions.

Matmul stages (gating, V, C, U) run in fp8-e4m3 DoubleRow mode. The 4
batch chunks (512 cols each) form 2 pairs; within a pair every matmul
is issued twice back-to-back with the same stationary tensor (A then
B), and a post-compile pass (_dedupe_ldweights) deletes the second,
redundant PE weight load -- the dominant per-pass overhead (~107ns on
the bottleneck engine). To keep those pairs adjacent through the Tile
scheduler, each pair of matmul outputs shares ONE [P, 2, CHUNK] PSUM
tile (A in the first bank, B in the second), so both halves become
ready together. All matmul outputs live in a single 4-slot rotation of
such tiles (8 banks); the small gating/softmax outputs are
partition-base-0 slices of the same rotation (hardware requires matmul
outputs to start at partition 0).

The two pairs are processed baseline-style, stage-staggered (pair 0's
full layer, then pair 1's), so one pair's ~20us of matmul work hides
the other pair's combine/softmax tails.

Residual reformulation: with S_i = sum_{j<i} (uv_j + b_j),
  xi_i = x0 (.) (S_i + 1).
Each layer's U matmuls accumulate uv into PSUM; for layers > 0 an
identity matmul adds the previous S (bf16, SBUF) into the same
accumulation (the PE absorbs the add -- the elementwise engines have
no spare capacity at this pace). Then one DVE scalar_tensor_tensor per
d-chunk emits xi = (S + (1 + B_i)) (.) x0 in fp8 for the next layer's
matmuls (bf16 on the last layer for the output DMA), and an ACT copy
spills S back to SBUF for the next layer. The tiny 1/sum copies run on
the otherwise idle Pool engine.
"""

import os
import sys

import ml_dtypes
import numpy as np

if "/opt/trn_rl_repo" not in sys.path:
    sys.path.insert(0, "/opt/trn_rl_repo")

import concourse.bass as bass
import concourse.bacc as bacc
import concourse.mybir as mybir
from concourse.tile import TileContext
from concourse.bass_utils import run_bass_kernel_spmd

AF = mybir.ActivationFunctionType
OP = mybir.AluOpType
DR = mybir.MatmulPerfMode.DoubleRow
F32 = mybir.dt.float32
WDT = mybir.dt.bfloat16
F8 = mybir.dt.float8e4
BF16 = ml_dtypes.bfloat16
NPF8 = ml_dtypes.float8_e4m3

N_CROSS = 3
E = 8            # experts
D = 1024         # feature dim
R = 64           # low rank
B = 16384        # full batch
NCORES = 8
BC = B // NCORES  # rows per core
CHUNK = 512       # batch tile (matmul free dim)
NCHUNK = BC // CHUNK
P = 128
KC = D // P       # d-chunks
ER = E * R        # 512
MC = ER // P      # (e,r)-chunks


def _dedupe_ldweights(nc):
    """Drop back-to-back redundant PE weight loads.

    Bacc's final codegen splits every matmul into Ldweights + Matmult.
    The pair-interleaved schedule issues consecutive matmuls with the
    same stationary tensor, so the second Ldweights re-loads what the
    PE array already holds. Remove any Ldweights identical to the
    previous one when only Matmult / EventSemaphore instructions ran on
    PE in between and it carries no semaphore waits or updates.
    """
    removed = 0
    for blk in nc.main_func.blocks:
        prev_sig = None
        dead = []
        for ins in blk.instructions:
            if ins.engine != mybir.EngineType.PE:
                continue
            if isinstance(ins, mybir.InstLdweights):
                a = ins.ins[0]
                sig = (a.memref, a.offset, tuple(map(tuple, a.ap)),
                       str(a.dtype), str(ins.perf_mode),
                       str(ins.is_transpose), str(ins.tile_position),
                       str(ins.tile_size))
                si = ins.sync_info
                clean = si is None or (len(si.on_wait) == 0
                                       and len(si.on_update) == 0)
                if clean and sig == prev_sig:
                    dead.append(ins)
                else:
                    prev_sig = sig
            elif isinstance(ins, mybir.InstMatmult):
                pass  # streaming does not disturb the loaded weights
            elif ins.opcode in ("EventSemaphore", "Nop"):
                pass
            else:
                prev_sig = None
        for ins in dead:
            blk.instructions.remove(ins)
        removed += len(dead)
    return removed


def _build():
    nc = bacc.Bacc(None)
    xT = nc.declare_dram_parameter("xT", [D, BC], WDT, isOutput=False)
    xT8 = nc.declare_dram_parameter("xT8", [D, BC], F8, isOutput=False)
    Vl = nc.declare_dram_parameter("Vl", [N_CROSS, D, ER], F8, isOutput=False)
    Cb = nc.declare_dram_parameter("Cb", [N_CROSS, MC, 2, P, P], F8, isOutput=False)
    Ul = nc.declare_dram_parameter("Ul", [N_CROSS, ER, D], F8, isOutput=False)
    # gating weights padded to 16 cols: DoubleRow lhsT outer step must be
    # a multiple of 16 (s3_lw dual-fp8 restriction)
    WgT = nc.declare_dram_parameter("WgT", [D, 2 * E], F8, isOutput=False)
    # bTc[i, p, kc] = 1 + sum_{j<=i} b[j, kc*P+p]  (per-partition stt scalar)
    bTc = nc.declare_dram_parameter("bTc", [N_CROSS, P, KC], F32, isOutput=False)
    sel = nc.declare_dram_parameter("sel", [E, MC + 1, P], WDT, isOutput=False)
    id128 = nc.declare_dram_parameter("id128", [P, P], WDT, isOutput=False)
    outT = nc.declare_dram_parameter("outT", [D, BC], WDT, isOutput=True)

    def mm_pair(pt, w, rhs_a, rhs_b, start, stop, perf_mode=None, rows=P):
        nc.tensor.matmul(pt[0:rows, 0, :], w, rhs_a, start=start, stop=stop,
                         perf_mode=perf_mode)
        nc.tensor.matmul(pt[0:rows, 1, :], w, rhs_b, start=start, stop=stop,
                         perf_mode=perf_mode)

    with TileContext(nc) as tc:
        with (
            tc.sbuf_pool(name="wpool", bufs=1) as wpool,
            tc.sbuf_pool(name="xpool", bufs=NCHUNK) as xpool,
            tc.sbuf_pool(name="x8pool", bufs=NCHUNK) as x8pool,
            tc.sbuf_pool(name="spool2", bufs=NCHUNK) as spool2,
            tc.sbuf_pool(name="xopool", bufs=2) as xopool,
            tc.sbuf_pool(name="h1pool", bufs=2) as h1pool,
            tc.sbuf_pool(name="h2pool", bufs=4) as h2pool,
            tc.sbuf_pool(name="ypool", bufs=2) as ypool,
            tc.sbuf_pool(name="spool", bufs=2) as spool,
            tc.psum_pool(name="psmm", bufs=4) as psmm,
        ):
            xTr = xT.rearrange("(kc p) b -> p kc b", p=P)
            xT8r = xT8.rearrange("(kc p) b -> p kc b", p=P)
            outr = outT.rearrange("(kc p) b -> p kc b", p=P)
            Vlr = Vl.rearrange("i (kc p) m -> p i kc m", p=P)
            Ulr = Ul.rearrange("i (mc p) d -> p i mc d", p=P)
            Cbr = Cb.rearrange("i m j p s -> p i m j s")

            def alloc_x(c):
                x0 = xpool.tile([P, KC, CHUNK], WDT, tag="x0", name=f"x0_{c}")
                s8 = x8pool.tile([P, KC, CHUNK], F8, tag="s8", name=f"s8_{c}")
                S = spool2.tile([P, KC, CHUNK], WDT, tag="S", name=f"S_{c}")
                return x0, s8, S

            def load_s8(tt, c):
                cbs = slice(c * CHUNK, (c + 1) * CHUNK)
                nc.sync.dma_start(tt[1], xT8r[:, :, cbs])

            def load_x0(tt, c):
                cbs = slice(c * CHUNK, (c + 1) * CHUNK)
                nc.sync.dma_start(tt[0], xTr[:, :, cbs])

            wg_sb = wpool.tile([P, KC, 2 * E], F8)
            nc.scalar.dma_start(wg_sb, WgT.rearrange("(kc p) e -> p kc e", p=P))

            v_sb = wpool.tile([P, N_CROSS, KC, ER], F8)
            u_sb = wpool.tile([P, N_CROSS, MC, D], F8)
            c_sb = wpool.tile([P, N_CROSS, MC, 2, P], F8)
            b_sb = wpool.tile([P, N_CROSS, KC], F32)
            id_sb = wpool.tile([P, P], WDT)

            nc.scalar.dma_start(v_sb[:, 0, 0:KC // 2], Vlr[:, 0, 0:KC // 2])
            nc.scalar.dma_start(v_sb[:, 0, KC // 2:], Vlr[:, 0, KC // 2:])
            nc.scalar.dma_start(c_sb, Cbr)

            # q1 in need-order: s8 for all 4 chunks (pair 1 starts its layer
            # right after pair 0's), U0, id, x0 c0..c3, U1, U2.
            tiles = {c: alloc_x(c) for c in range(NCHUNK)}
            load_s8(tiles[0], 0)
            load_s8(tiles[1], 1)
            nc.sync.dma_start(u_sb[:, 0], Ulr[:, 0])
            nc.sync.dma_start(id_sb, id128[:])
            load_s8(tiles[2], 2)
            load_s8(tiles[3], 3)
            load_x0(tiles[0], 0)
            sel_sb = wpool.tile([E, MC + 1, P], WDT)
            nc.scalar.dma_start(sel_sb, sel[:])
            nc.scalar.dma_start(v_sb[:, 1], Vlr[:, 1])
            load_x0(tiles[1], 1)
            nc.sync.dma_start(u_sb[:, 1], Ulr[:, 1])
            nc.scalar.dma_start(b_sb, bTc.rearrange("i p kc -> p i kc"))
            load_x0(tiles[2], 2)
            load_x0(tiles[3], 3)
            nc.scalar.dma_start(v_sb[:, 2], Vlr[:, 2])
            nc.sync.dma_start(u_sb[:, 2], Ulr[:, 2])

            ones_col = sel_sb[:, MC, 0:1]     # [E, 1] ones (sums lhsT)
            ones_row = sel_sb[0:1, MC, 0:E]   # [1, E] ones (wps lhsT)

            for i in range(N_CROSS):
                for pr in range(NCHUNK // 2):
                    ca, cb = 2 * pr, 2 * pr + 1
                    x0A, s8A, SA = tiles[ca]
                    x0B, s8B, SB = tiles[cb]
                    last = i == N_CROSS - 1
                    if last:
                        xoA = xopool.tile([P, KC, CHUNK], WDT, tag="xo",
                                          name=f"xoA_{pr}")
                        xoB = xopool.tile([P, KC, CHUNK], WDT, tag="xo",
                                          name=f"xoB_{pr}")
                    # ---- gating (fp8 DoubleRow, 16-col padded) ----
                    gps = psmm.tile([P, 2, CHUNK], F32, tag="mm")
                    for q in range(KC // 2):
                        mm_pair(gps,
                                wg_sb[:, 2 * q: 2 * q + 2, :],
                                s8A[:, 2 * q: 2 * q + 2, :],
                                s8B[:, 2 * q: 2 * q + 2, :],
                                start=(q == 0), stop=(q == KC // 2 - 1),
                                perf_mode=DR, rows=2 * E)
                    expA = spool.tile([E, CHUNK], WDT, tag="expA")
                    expB = spool.tile([E, CHUNK], WDT, tag="expB")
                    nc.scalar.activation(expA, gps[0:E, 0, :], AF.Exp)
                    nc.scalar.activation(expB, gps[0:E, 1, :], AF.Exp)
                    # ---- V stage (fp8 DoubleRow), softmax tail woven in ----
                    h1A = h1pool.tile([P, MC, CHUNK], F8, tag="h1A")
                    h1B = h1pool.tile([P, MC, CHUNK], F8, tag="h1B")
                    for mc in range(MC):
                        vps = psmm.tile([P, 2, CHUNK], F32, tag="mm")
                        for q in range(KC // 2):
                            mm_pair(vps,
                                    v_sb[:, i, 2 * q: 2 * q + 2,
                                         mc * P: (mc + 1) * P],
                                    s8A[:, 2 * q: 2 * q + 2, :],
                                    s8B[:, 2 * q: 2 * q + 2, :],
                                    start=(q == 0), stop=(q == KC // 2 - 1),
                                    perf_mode=DR)
                        nc.scalar.activation(h1A[:, mc, :], vps[:, 0, :],
                                             AF.Tanh)
                        nc.scalar.activation(h1B[:, mc, :], vps[:, 1, :],
                                             AF.Tanh)
                        if mc == 0:
                            # softmax denominators (PE waits on exp only)
                            sums = psmm.tile([P, 2, CHUNK], F32, tag="mm")
                            mm_pair(sums, ones_col, expA, expB,
                                    start=True, stop=True, rows=1)
                            rfA = spool.tile([1, CHUNK], F32, tag="rfA")
                            rfB = spool.tile([1, CHUNK], F32, tag="rfB")
                            nc.vector.reciprocal_approx_fast(rfA, sums[0:1, 0, :])
                            nc.vector.reciprocal_approx_fast(rfB, sums[0:1, 1, :])
                            rrA = spool.tile([1, CHUNK], WDT, tag="rrA")
                            rrB = spool.tile([1, CHUNK], WDT, tag="rrB")
                            nc.gpsimd.tensor_copy(rrA, rfA)
                            nc.gpsimd.tensor_copy(rrB, rfB)
                        if mc == 2:
                            # broadcast 1/sum to E partitions (rrow ready now)
                            wps = psmm.tile([P, 2, CHUNK], F32, tag="mm")
                            mm_pair(wps, ones_row, rrA, rrB,
                                    start=True, stop=True, rows=E)
                            wsbA = spool.tile([E, CHUNK], WDT, tag="wsbA")
                            wsbB = spool.tile([E, CHUNK], WDT, tag="wsbB")
                            nc.vector.tensor_tensor(wsbA, expA, wps[0:E, 0, :],
                                                    OP.mult)
                            nc.vector.tensor_tensor(wsbB, expB, wps[0:E, 1, :],
                                                    OP.mult)
                    # ---- C stage + gate broadcast + ys ----
                    ysA = ypool.tile([P, MC, CHUNK], F8, tag="ysA")
                    ysB = ypool.tile([P, MC, CHUNK], F8, tag="ysB")
                    for mc in range(MC):
                        qb = (mc // 2) * 2
                        cps = psmm.tile([P, 2, CHUNK], F32, tag="mm")
                        mm_pair(cps, c_sb[:, i, mc, :, :],
                                h1A[:, qb: qb + 2, :], h1B[:, qb: qb + 2, :],
                                start=True, stop=True, perf_mode=DR)
                        wbp = psmm.tile([P, 2, CHUNK], F32, tag="mm")
                        mm_pair(wbp, sel_sb[:, mc, :], wsbA, wsbB,
                                start=True, stop=True)
                        h2A = h2pool.tile([P, CHUNK], F32, tag="h2")
                        h2B = h2pool.tile([P, CHUNK], F32, tag="h2")
                        nc.scalar.activation(h2A, cps[:, 0, :], AF.Tanh)
                        nc.scalar.activation(h2B, cps[:, 1, :], AF.Tanh)
                        nc.vector.tensor_tensor(ysA[:, mc, :], h2A,
                                                wbp[:, 0, :], OP.mult)
                        nc.vector.tensor_tensor(ysB[:, mc, :], h2B,
                                                wbp[:, 1, :], OP.mult)
                    # ---- U stage + S accumulate (PE) + combine ----
                    for dc in range(KC):
                        ups = psmm.tile([P, 2, CHUNK], F32, tag="mm")
                        for q in range(MC // 2):
                            mm_pair(ups,
                                    u_sb[:, i, 2 * q: 2 * q + 2,
                                         dc * P: (dc + 1) * P],
                                    ysA[:, 2 * q: 2 * q + 2, :],
                                    ysB[:, 2 * q: 2 * q + 2, :],
                                    start=(q == 0),
                                    stop=(i == 0 and q == MC // 2 - 1),
                                    perf_mode=DR)
                        if i > 0:
                            mm_pair(ups, id_sb, SA[:, dc, :], SB[:, dc, :],
                                    start=False, stop=True)
                        # xi = (S + (1 + B_i)) * x0
                        dlt = b_sb[:, i, dc: dc + 1]
                        outA = (xoA if last else s8A)[:, dc, :]
                        outB = (xoB if last else s8B)[:, dc, :]
                        nc.vector.scalar_tensor_tensor(
                            outA, ups[:, 0, :], dlt, x0A[:, dc, :],
                            OP.add, OP.mult)
                        nc.vector.scalar_tensor_tensor(
                            outB, ups[:, 1, :], dlt, x0B[:, dc, :],
                            OP.add, OP.mult)
                        if not last:
                            nc.scalar.activation(SA[:, dc, :], ups[:, 0, :],
                                                 AF.Copy)
                            nc.scalar.activation(SB[:, dc, :], ups[:, 1, :],
                                                 AF.Copy)
                        else:
                            bsA = slice(ca * CHUNK, (ca + 1) * CHUNK)
                            bsB = slice(cb * CHUNK, (cb + 1) * CHUNK)
                            eng = nc.sync if dc % 2 == 0 else nc.scalar
                            eng.dma_start(outr[:, dc, bsA], xoA[:, dc, :])
                            eng2 = nc.scalar if dc % 2 == 0 else nc.sync
                            eng2.dma_start(outr[:, dc, bsB], xoB[:, dc, :])
    nc.compile()
    _dedupe_ldweights(nc)
    return nc


_CTX = {}


def _get_nc():
    if "nc" not in _CTX:
        _CTX["nc"] = _build()
    return _CTX["nc"]


def _prep_weights(U, V, C, Wg, b):
    f = np.float32
    U = np.asarray(U, dtype=f)
    V = np.asarray(V, dtype=f)
    C = np.asarray(C, dtype=f)
    Wg = np.asarray(Wg, dtype=f)
    b = np.asarray(b, dtype=f)
    # Vl[i, d, e*R+r] = V[i, e, d, r]
    Vl = np.ascontiguousarray(V.transpose(0, 2, 1, 3).reshape(N_CROSS, D, ER))
    # Ul[i, e*R+r, d] = U[i, e, d, r]
    Ul = np.ascontiguousarray(U.transpose(0, 1, 3, 2).reshape(N_CROSS, ER, D))
    # DoubleRow C: out-block mc pairs rhs h1 blocks (qb, qb+1); the plane
    # matching block mc carries the block-diag expert pair, the other is 0.
    Cb2 = np.zeros((N_CROSS, MC, 2, P, P), dtype=f)
    for i in range(N_CROSS):
        for m in range(MC):
            blk = np.zeros((P, P), dtype=f)
            blk[:R, :R] = C[i, 2 * m]
            blk[R:, R:] = C[i, 2 * m + 1]
            Cb2[i, m, m % 2] = blk
    WgT = np.zeros((D, 2 * E), dtype=f)
    WgT[:, :E] = Wg.T
    # bTc[i, p, kc] = 1 + cumsum_i b  (stt per-partition scalar)
    bc = 1.0 + np.cumsum(b, axis=0)
    bTc = np.ascontiguousarray(bc.reshape(N_CROSS, KC, P).transpose(0, 2, 1))
    sel = np.zeros((E, MC + 1, P), dtype=f)
    for m in range(MC):
        for j in range(P):
            sel[2 * m + j // R, m, j] = 1.0
    sel[:, MC, :] = 1.0
    return dict(
        Vl=Vl.astype(NPF8),
        Ul=Ul.astype(NPF8),
        Cb=Cb2.astype(NPF8),
        WgT=WgT.astype(NPF8),
        bTc=bTc,
        sel=sel.astype(BF16),
        id128=np.eye(P, dtype=f).astype(BF16),
    )


def kernel(x, U, V, C, Wg, b, _trace=False):
    nc = _get_nc()
    w = _prep_weights(U, V, C, Wg, b)
    xs = np.asarray(x, dtype=np.float32).reshape(NCORES, BC, D)
    in_maps = []
    for ci in range(NCORES):
        xt = np.ascontiguousarray(xs[ci].T)
        m = {"xT": xt.astype(BF16), "xT8": xt.astype(NPF8)}
        m.update(w)
        in_maps.append(m)
    res = run_bass_kernel_spmd(nc, in_maps, list(range(NCORES)), trace=_trace)
    kernel.last_result = res
    out = np.concatenate(
        [np.asarray(res.results[ci]["outT"]).astype(np.float32).T
         for ci in range(NCORES)],
        axis=0,
    )
    return np.ascontiguousarray(out, dtype=np.float32)
